# revision 40
# baseline (speedup 1.0000x reference)
"""Trainium2 Bass kernel for a 12-head attention block.

Problem (hardcoded): x [16, 1024, 768] f32, w_qkv [2304, 768], w_proj
[768, 768], b_proj [768].  out = proj(softmax(q k^T / sqrt(64)) v).

Sharding: pure data parallel over batch - 16 batches / 8 cores = 2
batches per core, no collectives.  All layout transposes happen on the
host: each core receives x^T per batch and produces out^T per batch.

v2 design (vs the f32r baseline):
  * all matmul operands in bf16 (same 1 cycle/row PE speed as f32r but
    half the DMA bytes and half the PSUM->SBUF copy cost); PSUM stays
    f32 so accumulation precision is unchanged.
  * cross-batch software pipelining: batch b+1's qkv projection matmuls
    are emitted interleaved between the attention heads of batch b, and
    batch 0's output projection is interleaved into batch 1's attention.
    The PE queue therefore always has ready work, which both hides the
    exp (ACT) latency and keeps the PE p-state at full clock.
  * cross-head score prefetch: the first QK^T chunk of head h+1 is
    issued while head h's last PV is still waiting on its exp.
  * PV accumulators are split into two [65, 512] half-tiles so PSUM
    fits: 2x s-tile [128,1024] (4 banks) + 1x phase-A/proj [128,1024]
    (2 banks) + 2x o_ps [65,512] (2 banks) = 8 banks.

Per-core per-batch structure (all moving dims 512 where possible):
  A) qkv projection: q^T,k^T per head-pair chunk [128, 1024] bf16; V in
     natural [j, d] layout per j-chunk, ones-augmented ([128, 12, 65]).
  B) per head: S^T[j,i] chunks -> exp on ACT (scale folded, bf16 out)
     -> O^T_aug[65, i] halves accumulated over j-chunks in PSUM.
     Row 64 of O^T_aug is the softmax denominator l[i].
  C) normalize per half: r = 1/l on DVE, broadcast r across 64
     partitions on GPSIMD, multiply on DVE (bf16 out).
  D) proj: out^T = w_proj^T.T @ O^T + b_proj, DMA back to DRAM (f32).
"""

import numpy as np
from contextlib import ExitStack

import ml_dtypes

import concourse.bass as bass
import concourse.mybir as mybir
import concourse.tile as tile
from concourse import bacc
from concourse import bass_utils

F32 = mybir.dt.float32
BF16 = mybir.dt.bfloat16
FP8 = mybir.dt.float8e4
EXP = mybir.ActivationFunctionType.Exp

B, N, C = 16, 1024, 768
H, D = 12, 64
E = 3 * C
NCORES = 8
BL = B // NCORES          # batches per core
T = BL * N                # tokens per core
KC = C // 128             # feature chunks of 128
JC = N // 128             # token chunks of 128
SCALE = float(D) ** -0.5

_CACHE = {}


def _mm(nc, out, lhsT, rhs, **kw):
    nc.tensor.matmul(out, lhsT=lhsT, rhs=rhs, **kw)


def _build(ctx, tc):
    nc = tc.nc
    dram = ctx.enter_context(tc.tile_pool(name="dram", bufs=1, space="DRAM"))
    # x^T blocked: [kc, b, 128, N] bf16, each per-batch chunk contiguous
    xT_d = dram.tile([KC, BL, 128, N], BF16, kind="ExternalInput", name="xTb", uniquify=False)
    # q/k weights j-major: [j, p, kc*128+c] so one j-output-chunk (used by
    # one qk_unit across all kc) is a single contiguous-row DMA
    # fp8 with a x64 host-side pre-scale (raw weights are subnormal in
    # e4m3); the 1/4096 comes back out inside the exp's scale argument
    wqkj_d = dram.tile([2 * KC, 128, C], FP8, kind="ExternalInput", name="wqkjb", uniquify=False)
    # v weights per-kc slabs [kc, 128, 768] bf16
    wv_d = dram.tile([KC, 128, C], BF16, kind="ExternalInput", name="wvb", uniquify=False)
    # w_proj^T per-kc slabs [kc, 128, 768] bf16
    wproj_d = dram.tile([KC, 128, C], BF16, kind="ExternalInput", name="wprojb", uniquify=False)
    bproj_d = dram.tile([C, 1], F32, kind="ExternalInput", name="bproj", uniquify=False)
    # out^T blocked: [oc, b, 128, N] f32
    outT_d = dram.tile([KC, BL, 128, N], F32, kind="ExternalOutput", name="outTb", uniquify=False)

    consts = ctx.enter_context(tc.tile_pool(name="consts", bufs=1))
    wqk_pool = ctx.enter_context(tc.tile_pool(name="wqk", bufs=2 * KC))
    wv_pool = ctx.enter_context(tc.tile_pool(name="wv", bufs=KC))
    wp_pool = ctx.enter_context(tc.tile_pool(name="wproj", bufs=KC))
    xt_pool = ctx.enter_context(tc.tile_pool(name="xt", bufs=2 * KC))
    qk_pool = ctx.enter_context(tc.tile_pool(name="qk", bufs=4 * KC))
    va_pool = ctx.enter_context(tc.tile_pool(name="va", bufs=2 * JC))
    ot_pool = ctx.enter_context(tc.tile_pool(name="ot", bufs=2 * KC))
    p_pool = ctx.enter_context(tc.tile_pool(name="pt", bufs=3))
    sm_pool = ctx.enter_context(tc.tile_pool(name="small", bufs=4))
    lb_pool = ctx.enter_context(tc.tile_pool(name="lb", bufs=2))
    ob_pool = ctx.enter_context(tc.tile_pool(name="ob", bufs=2))
    ps_s = ctx.enter_context(tc.tile_pool(name="ps_s", bufs=2, space="PSUM"))
    # phase-A / out-proj accumulators as [128, 512] HALF tiles, double
    # buffered (1 bank each): a unit's second half runs while the first
    # half drains on DVE, so the in-order PE queue never waits on a drain.
    ps_a = ctx.enter_context(tc.tile_pool(name="ps_a", bufs=2, space="PSUM"))
    po_pool = ctx.enter_context(tc.tile_pool(name="po", bufs=2, space="PSUM"))

    bias_sb = consts.tile([128, KC], F32)
    nc.sync.dma_start(
        out=bias_sb, in_=bproj_d[:, 0].rearrange("(k p) -> p k", p=128)
    )

    # ---- input DMA, ordered by first use.  Each dma_start costs ~650ns
    # of serial issue time on the Sync engine, so the critical first few
    # transfers are kept big and few: k0's weights (one j-major tile), x,
    # v weights, then the remaining j-tiles in unit order. ----
    xt = {}   # (b, kc) -> [128, N] bf16
    wqk_t, wv = {}, []

    def load_wqkj(j, split=1):
        t = wqk_pool.tile([128, KC, 128], FP8, name=f"wqkj{j}", tag="wqk")
        step = 128 // split
        for p in range(0, 128, step):
            nc.sync.dma_start(
                out=t[p:p + step, :, :].rearrange("p a b -> p (a b)"),
                in_=wqkj_d[j, p:p + step, :])
        wqk_t[j] = t

    load_wqkj(KC)      # k weights for mt=0: the very first unit
    for kc in range(KC):
        xt[(0, kc)] = xt_pool.tile([128, N], BF16, name=f"xt0_{kc}", tag="xt")
        for p in range(0, 128, 64):
            nc.sync.dma_start(out=xt[(0, kc)][p:p + 64, :],
                              in_=xT_d[kc, 0, p:p + 64, :])
    load_wqkj(0)       # q weights for mt=0: the second unit
    for kc in range(KC):
        wvt = wv_pool.tile([128, KC, 128], BF16, name=f"wv{kc}", tag="wv")
        nc.sync.dma_start(out=wvt.rearrange("p a b -> p (a b)"), in_=wv_d[kc])
        wv.append(wvt)
    for mt in range(1, KC):
        load_wqkj(KC + mt)
        load_wqkj(mt)
    wqk = {(j, kc): wqk_t[j][:, kc, :] for j in range(2 * KC) for kc in range(KC)}
    wp = {}
    for kc in range(KC):
        t = wp_pool.tile([128, KC, 128], BF16, name=f"wpk{kc}", tag="wp")
        nc.sync.dma_start(out=t.rearrange("p a b -> p (a b)"), in_=wproj_d[kc])
        for oc in range(KC):
            wp[(kc, oc)] = t[:, oc, :]
    # batch-1 x after the weights (needed ~60us in, arrives in ~15us)
    for kc in range(KC):
        xt[(1, kc)] = xt_pool.tile([128, N], BF16, name=f"xt1_{kc}", tag="xt")
        nc.sync.dma_start(out=xt[(1, kc)], in_=xT_d[kc, 1])

    qt = {}   # (b, mt) -> [128, N] bf16   q features, head-pair mt
    kt = {}   # (b, mt)
    va = {}   # (b, jc) -> [128, H, D+1] bf16
    ot = {}   # (b, mt) -> [128, N] bf16

    # ---- phase A units: qkv projection for batch b, as closures ----
    def phase_a_units(b):
        units = {}

        def v_unit(jc):
            def emit():
                vat = va_pool.tile([128, H, D + 1], BF16, name=f"va{b}_{jc}", tag="va")
                xs = [xt[(b, kc)][:, jc * 128:(jc + 1) * 128] for kc in range(KC)]
                wvf = [wv[kc].rearrange("p a b -> p (a b)") for kc in range(KC)]
                vps_a = ps_a.tile([128, 512], F32, name=f"vpsa{b}_{jc}", tag="a")
                for kc in range(KC):
                    _mm(nc, vps_a, xs[kc], wvf[kc][:, 0:512],
                        start=(kc == 0), stop=(kc == KC - 1))
                nc.vector.tensor_copy(
                    out=vat[:, 0:8, 0:D],
                    in_=vps_a.rearrange("p (h d) -> p h d", h=8),
                )
                vps_b = ps_a.tile([128, 512], F32, name=f"vpsb{b}_{jc}", tag="a")
                for kc in range(KC):
                    _mm(nc, vps_b[:, 0:256], xs[kc], wvf[kc][:, 512:C],
                        start=(kc == 0), stop=(kc == KC - 1))
                nc.vector.tensor_copy(
                    out=vat[:, 8:H, 0:D],
                    in_=vps_b[:, 0:256].rearrange("p (h d) -> p h d", h=4),
                )
                nc.vector.memset(vat[:, :, D:D + 1], 1.0)
                va[(b, jc)] = vat
            return emit

        def qk_unit(which, mt):
            def emit():
                # k (which=1) is only ever a stationary operand of QK^T:
                # store it as fp8e4 so the per-matmul weight load halves.
                # One-sided k quantization error (~2.5% rms) only perturbs
                # the softmax logits by ~0.006 abs -> ~0.6% on P.
                dt = FP8 if which == 1 else BF16
                dest = qk_pool.tile([128, N], dt, name=f"qk{b}_{which}_{mt}", tag="qk")
                for hf in range(2):
                    ps = ps_a.tile([128, 512], F32, name=f"ps{b}_{which}_{mt}_{hf}", tag="a")
                    for kc in range(KC):
                        w = wqk[(which * KC + mt, kc)]
                        _mm(nc, ps,
                            w, xt[(b, kc)][:, hf * 512:(hf + 1) * 512],
                            start=(kc == 0), stop=(kc == KC - 1))
                    nc.vector.tensor_copy(out=dest[:, hf * 512:(hf + 1) * 512], in_=ps)
                (qt if which == 0 else kt)[(b, mt)] = dest
            return emit

        for jc in range(JC):
            units[("v", jc)] = v_unit(jc)
        for mt in range(KC):
            units[("k", mt)] = qk_unit(1, mt)
            units[("q", mt)] = qk_unit(0, mt)
        return units

    # ---- attention for batch b, with filler units paced between heads ----
    def qkt(b, h, jc):
        mt, off = h // 2, (h % 2) * D
        s = ps_s.tile([128, N], F32, name=f"s{b}_{h}_{jc}", tag="s")
        for hf in range(2):
            _mm(nc, s[:, hf * 512:(hf + 1) * 512],
                kt[(b, mt)][off:off + D, jc * 128:(jc + 1) * 128],
                qt[(b, mt)][off:off + D, hf * 512:(hf + 1) * 512])
        return s

    def attention(b, fillers):
        # fillers: list of (deadline_point, fn).  Units are emitted in list
        # order, spread evenly over the 8*H per-jc pace points; a unit
        # whose deadline arrives is forced out (with everything before it).
        # CRITICAL: pace() is called BEFORE each PV emission - the PE queue
        # is in-order, so filler matmuls must sit in front of the PV that
        # waits on its exp, or they cannot cover the ACT latency.
        emitted = 0
        npace = JC * H

        def pace(point):
            nonlocal emitted
            due = ((point + 1) * len(fillers) + npace - 1) // npace
            while emitted < len(fillers) and (
                emitted < due or fillers[emitted][0] <= point
            ):
                fillers[emitted][1]()
                emitted += 1

        s = qkt(b, 0, 0)
        for h in range(H):
            mt, off = h // 2, (h % 2) * D
            o_ps = [po_pool.tile([D + 1, 512], F32, name=f"ops{b}_{h}_{hf}", tag="ops")
                    for hf in range(2)]
            for jc in range(JC):
                pt = p_pool.tile([128, N], BF16, name=f"pt{b}_{h}_{jc}", tag="pt")
                nc.scalar.activation(out=pt, in_=s, func=EXP, scale=SCALE / 4096.0)
                if jc + 1 < JC:
                    s = qkt(b, h, jc + 1)
                elif h + 1 < H:
                    s = qkt(b, h + 1, 0)
                pace(JC * h + jc)
                for hf in range(2):
                    _mm(nc, o_ps[hf],
                        va[(b, jc)][:, h, :], pt[:, hf * 512:(hf + 1) * 512],
                        start=(jc == 0), stop=(jc == JC - 1))
            # normalize halves: rows 0..63 divided by l (= row 64)
            if (b, mt) not in ot:
                ot[(b, mt)] = ot_pool.tile([128, N], BF16, name=f"ot{b}_{mt}", tag="ot")
            # interleave the two halves' chains so the gpsimd broadcast of
            # half 0 overlaps the DVE copy/recip of half 1
            l_sbs, lbs = [], []
            for hf in range(2):
                l_sb = sm_pool.tile([1, 512], F32, name=f"l{b}_{h}_{hf}", tag="sm")
                nc.vector.tensor_copy(out=l_sb, in_=o_ps[hf][D:D + 1, :])
                nc.vector.reciprocal_approx_fast(out=l_sb, in_=l_sb)
                l_sbs.append(l_sb)
                lb = lb_pool.tile([D, 512], F32, name=f"lb{b}_{h}_{hf}", tag="lb")
                nc.gpsimd.partition_broadcast(lb, l_sb, channels=D)
                lbs.append(lb)
            for hf in range(2):
                nc.vector.tensor_mul(
                    out=ot[(b, mt)][off:off + D, hf * 512:(hf + 1) * 512],
                    in0=o_ps[hf][0:D, :], in1=lbs[hf],
                )

    # ---- output projection units for batch b (one unit per oc-half, so
    # the filler granularity is ~1.4us and every head boundary gets one) ----
    def outproj_units(b):
        units = []
        obs = {}

        def half_unit(oc, hf):
            def emit():
                if oc not in obs:
                    obs[oc] = ob_pool.tile([128, N], F32, name=f"ob{b}_{oc}", tag="ob")
                pps = ps_a.tile([128, 512], F32, name=f"pps{b}_{oc}_{hf}", tag="a")
                for kc in range(KC):
                    _mm(nc, pps,
                        wp[(kc, oc)],
                        ot[(b, kc)][:, hf * 512:(hf + 1) * 512],
                        start=(kc == 0), stop=(kc == KC - 1))
                nc.vector.tensor_scalar_add(
                    out=obs[oc][:, hf * 512:(hf + 1) * 512],
                    in0=pps, scalar1=bias_sb[:, oc:oc + 1])
                if hf == 1:
                    nc.sync.dma_start(out=outT_d[oc, b], in_=obs[oc])
            return emit

        for oc in range(KC):
            for hf in range(2):
                units.append(half_unit(oc, hf))
        return units

    # ---- pipeline ----
    # Window balance: the PE work of batch-1 qkv-proj and batch-0 out-proj
    # is spread across both attention windows so each window's PE work
    # slightly exceeds its ACT (exp) work and the PE never goes idle.
    # qk units for head-pair mt are consumed by qkt(h=2mt, 0), prefetched
    # at pace point 2*(2mt-1)+1; deadline one point earlier.
    a0 = phase_a_units(0)
    a1 = phase_a_units(1)
    LAST = JC * H - 1
    for key in [("k", 0), ("q", 0)] + [("v", jc) for jc in range(JC)] + [
        ("k", 1), ("q", 1), ("k", 2), ("q", 2), ("k", 3), ("q", 3)
    ]:
        a0[key]()
    fill0 = []
    for mt in (4, 5):  # batch-0 tail projections, hard deadlines
        dl = JC * (2 * mt - 1) - 1
        fill0 += [(dl, a0[("k", mt)]), (dl, a0[("q", mt)])]
    fill0 += [(LAST, a1[("v", jc)]) for jc in range(JC)]
    fill0 += [(LAST, a1[(w, 0)]) for w in ("k", "q")]
    attention(0, fill0)
    fill1 = []
    for mt in (1, 2, 3, 4, 5):
        dl = max(JC * (2 * mt - 1) - 1, 0)
        fill1 += [(dl, a1[("k", mt)]), (dl, a1[("q", mt)])]
    fill1 += [(LAST, u) for u in outproj_units(0)]
    attention(1, fill1)
    for u in outproj_units(1):
        u()


def get_nc():
    if "nc" not in _CACHE:
        nc = bacc.Bacc(None, target_bir_lowering=False, debug=False)
        with tile.TileContext(nc) as tc:
            with ExitStack() as ctx:
                _build(ctx, tc)
        nc.compile()
        _CACHE["nc"] = nc
    return _CACHE["nc"]


def make_in_maps(x, w_qkv, w_proj, b_proj):
    x = np.asarray(x, dtype=np.float32)
    w_qkv = np.asarray(w_qkv, dtype=np.float32)
    w_proj = np.asarray(w_proj, dtype=np.float32)
    # q/k weights j-major: wqkjb[j, p, kc*128+c2] = w_qkv[j*128+c2, kc*128+p]
    # x64 pre-scale to land the values in fp8e4's normal range
    wqkj = np.ascontiguousarray(
        w_qkv[0:2 * C].reshape(2 * KC, 128, KC, 128).transpose(0, 3, 2, 1)
        .reshape(2 * KC, 128, C) * 64.0
    ).astype(ml_dtypes.float8_e4m3)
    # v weights per-kc slabs [kc, 128p, 768]: w_qkv^T[c, 2C:3C]
    wvb = np.ascontiguousarray(
        w_qkv[2 * C:3 * C].T.reshape(KC, 128, C)
    ).astype(ml_dtypes.bfloat16)
    # w_proj^T [c, o] -> per-kc slabs [kc, 128, 768] bf16
    wprojb = np.ascontiguousarray(w_proj.T.reshape(KC, 128, C)).astype(ml_dtypes.bfloat16)
    bp = np.ascontiguousarray(b_proj.astype(np.float32).reshape(C, 1))
    in_maps = []
    for c in range(NCORES):
        # x^T [c, t] -> blocks [kc, b, 128, N] bf16
        xT = x[c * BL:(c + 1) * BL].reshape(T, C).T  # [768, 2048]
        xb = np.ascontiguousarray(
            xT.reshape(KC, 128, BL, N).transpose(0, 2, 1, 3)
        ).astype(ml_dtypes.bfloat16)
        in_maps.append({"xTb": xb, "wqkjb": wqkj, "wvb": wvb,
                        "wprojb": wprojb, "bproj": bp})
    return in_maps


def assemble_out(results):
    outs = []
    for c in range(NCORES):
        ob = results[c]["outTb"]  # [oc, b, 128, N]
        oT = ob.transpose(0, 2, 1, 3).reshape(C, T)
        outs.append(np.ascontiguousarray(oT.T).reshape(BL, N, C))
    return np.concatenate(outs, axis=0).astype(np.float32)


def kernel(x, w_qkv, w_proj, b_proj):
    nc = get_nc()
    in_maps = make_in_maps(x, w_qkv, w_proj, b_proj)
    res = bass_utils.run_bass_kernel_spmd(nc, in_maps, core_ids=list(range(NCORES)))
    return assemble_out(res.results)


# revision 42
# speedup vs baseline: 1.0555x; 1.0555x over previous
"""Trainium2 Bass kernel for a 12-head attention block.

Problem (hardcoded): x [16, 1024, 768] f32, w_qkv [2304, 768], w_proj
[768, 768], b_proj [768].  out = proj(softmax(q k^T / sqrt(64)) v).

Sharding: pure data parallel over batch - 16 batches / 8 cores = 2
batches per core, no collectives.  All layout transposes happen on the
host: each core receives x^T per batch and produces out^T per batch.

v2 design (vs the f32r baseline):
  * all matmul operands in bf16 (same 1 cycle/row PE speed as f32r but
    half the DMA bytes and half the PSUM->SBUF copy cost); PSUM stays
    f32 so accumulation precision is unchanged.
  * cross-batch software pipelining: batch b+1's qkv projection matmuls
    are emitted interleaved between the attention heads of batch b, and
    batch 0's output projection is interleaved into batch 1's attention.
    The PE queue therefore always has ready work, which both hides the
    exp (ACT) latency and keeps the PE p-state at full clock.
  * cross-head score prefetch: the first QK^T chunk of head h+1 is
    issued while head h's last PV is still waiting on its exp.
  * PV accumulators are split into two [65, 512] half-tiles so PSUM
    fits: 2x s-tile [128,1024] (4 banks) + 1x phase-A/proj [128,1024]
    (2 banks) + 2x o_ps [65,512] (2 banks) = 8 banks.

Per-core per-batch structure (all moving dims 512 where possible):
  A) qkv projection: q^T,k^T per head-pair chunk [128, 1024] bf16; V in
     natural [j, d] layout per j-chunk, ones-augmented ([128, 12, 65]).
  B) per head: S^T[j,i] chunks -> exp on ACT (scale folded, bf16 out)
     -> O^T_aug[65, i] halves accumulated over j-chunks in PSUM.
     Row 64 of O^T_aug is the softmax denominator l[i].
  C) normalize per half: r = 1/l on DVE, broadcast r across 64
     partitions on GPSIMD, multiply on DVE (bf16 out).
  D) proj: out^T = w_proj^T.T @ O^T + b_proj, DMA back to DRAM (f32).
"""

import numpy as np
from contextlib import ExitStack

import ml_dtypes

import concourse.bass as bass
import concourse.mybir as mybir
import concourse.tile as tile
from concourse import bacc
from concourse import bass_utils

F32 = mybir.dt.float32
BF16 = mybir.dt.bfloat16
FP8 = mybir.dt.float8e4
EXP = mybir.ActivationFunctionType.Exp

B, N, C = 16, 1024, 768
H, D = 12, 64
E = 3 * C
NCORES = 8
BL = B // NCORES          # batches per core
T = BL * N                # tokens per core
KC = C // 128             # feature chunks of 128
JC = N // 128             # token chunks of 128
SCALE = float(D) ** -0.5

_CACHE = {}


def _mm(nc, out, lhsT, rhs, **kw):
    nc.tensor.matmul(out, lhsT=lhsT, rhs=rhs, **kw)


def _build(ctx, tc):
    nc = tc.nc
    dram = ctx.enter_context(tc.tile_pool(name="dram", bufs=1, space="DRAM"))
    # x^T blocked: [kc, b, 128, N] bf16, each per-batch chunk contiguous
    xT_d = dram.tile([KC, BL, 128, N], BF16, kind="ExternalInput", name="xTb", uniquify=False)
    # q/k weights j-major: [j, p, kc*128+c] so one j-output-chunk (used by
    # one qk_unit across all kc) is a single contiguous-row DMA
    wqkj_d = dram.tile([2 * KC, 128, C], BF16, kind="ExternalInput", name="wqkjb", uniquify=False)
    # v weights per-kc slabs [kc, 128, 768] bf16
    wv_d = dram.tile([KC, 128, C], BF16, kind="ExternalInput", name="wvb", uniquify=False)
    # w_proj^T per-kc slabs [kc, 128, 768] bf16
    wproj_d = dram.tile([KC, 128, C], BF16, kind="ExternalInput", name="wprojb", uniquify=False)
    bproj_d = dram.tile([C, 1], F32, kind="ExternalInput", name="bproj", uniquify=False)
    # out^T blocked: [oc, b, 128, N] f32
    outT_d = dram.tile([KC, BL, 128, N], F32, kind="ExternalOutput", name="outTb", uniquify=False)

    consts = ctx.enter_context(tc.tile_pool(name="consts", bufs=1))
    wqk_pool = ctx.enter_context(tc.tile_pool(name="wqk", bufs=2 * KC))
    wv_pool = ctx.enter_context(tc.tile_pool(name="wv", bufs=KC))
    wp_pool = ctx.enter_context(tc.tile_pool(name="wproj", bufs=KC))
    xt_pool = ctx.enter_context(tc.tile_pool(name="xt", bufs=2 * KC))
    qk_pool = ctx.enter_context(tc.tile_pool(name="qk", bufs=4 * KC))
    va_pool = ctx.enter_context(tc.tile_pool(name="va", bufs=2 * JC))
    ot_pool = ctx.enter_context(tc.tile_pool(name="ot", bufs=2 * KC))
    p_pool = ctx.enter_context(tc.tile_pool(name="pt", bufs=4))
    sm_pool = ctx.enter_context(tc.tile_pool(name="small", bufs=8))
    lb_pool = ctx.enter_context(tc.tile_pool(name="lb", bufs=4))
    ob_pool = ctx.enter_context(tc.tile_pool(name="ob", bufs=2))
    ps_s = ctx.enter_context(tc.tile_pool(name="ps_s", bufs=2, space="PSUM"))
    # phase-A / out-proj accumulators as [128, 512] HALF tiles, double
    # buffered (1 bank each): a unit's second half runs while the first
    # half drains on DVE, so the in-order PE queue never waits on a drain.
    ps_a = ctx.enter_context(tc.tile_pool(name="ps_a", bufs=2, space="PSUM"))
    po_pool = ctx.enter_context(tc.tile_pool(name="po", bufs=2, space="PSUM"))

    bias_sb = consts.tile([128, KC], F32)
    nc.sync.dma_start(
        out=bias_sb, in_=bproj_d[:, 0].rearrange("(k p) -> p k", p=128)
    )

    # ---- input DMA, ordered by first use.  Each dma_start costs ~650ns
    # of serial issue time on the Sync engine, so the critical first few
    # transfers are kept big and few: k0's weights (one j-major tile), x,
    # v weights, then the remaining j-tiles in unit order. ----
    xt = {}   # (b, kc) -> [128, N] bf16
    wqk_t, wv = {}, []

    def load_wqkj(j, split=1):
        t = wqk_pool.tile([128, KC, 128], BF16, name=f"wqkj{j}", tag="wqk")
        step = 128 // split
        for p in range(0, 128, step):
            nc.sync.dma_start(
                out=t[p:p + step, :, :].rearrange("p a b -> p (a b)"),
                in_=wqkj_d[j, p:p + step, :])
        wqk_t[j] = t

    load_wqkj(KC)      # k weights for mt=0: the very first unit
    for kc in range(KC):
        xt[(0, kc)] = xt_pool.tile([128, N], BF16, name=f"xt0_{kc}", tag="xt")
        for p in range(0, 128, 64):
            nc.sync.dma_start(out=xt[(0, kc)][p:p + 64, :],
                              in_=xT_d[kc, 0, p:p + 64, :])
    load_wqkj(0)       # q weights for mt=0: the second unit
    for kc in range(KC):
        wvt = wv_pool.tile([128, KC, 128], BF16, name=f"wv{kc}", tag="wv")
        nc.sync.dma_start(out=wvt.rearrange("p a b -> p (a b)"), in_=wv_d[kc])
        wv.append(wvt)
    for mt in range(1, KC):
        load_wqkj(KC + mt)
        load_wqkj(mt)
    wqk = {(j, kc): wqk_t[j][:, kc, :] for j in range(2 * KC) for kc in range(KC)}
    wp = {}
    for kc in range(KC):
        t = wp_pool.tile([128, KC, 128], BF16, name=f"wpk{kc}", tag="wp")
        nc.sync.dma_start(out=t.rearrange("p a b -> p (a b)"), in_=wproj_d[kc])
        for oc in range(KC):
            wp[(kc, oc)] = t[:, oc, :]
    # batch-1 x after the weights (needed ~60us in, arrives in ~15us)
    for kc in range(KC):
        xt[(1, kc)] = xt_pool.tile([128, N], BF16, name=f"xt1_{kc}", tag="xt")
        nc.sync.dma_start(out=xt[(1, kc)], in_=xT_d[kc, 1])

    qt = {}   # (b, mt) -> [128, N] bf16   q features, head-pair mt
    kt = {}   # (b, mt)
    va = {}   # (b, jc) -> [128, H, D+1] bf16
    ot = {}   # (b, mt) -> [128, N] bf16

    # ---- phase A units: qkv projection for batch b, as closures ----
    def phase_a_units(b):
        units = {}

        def v_unit(jc):
            def emit():
                vat = va_pool.tile([128, H, D + 1], BF16, name=f"va{b}_{jc}", tag="va")
                xs = [xt[(b, kc)][:, jc * 128:(jc + 1) * 128] for kc in range(KC)]
                wvf = [wv[kc].rearrange("p a b -> p (a b)") for kc in range(KC)]
                vps_a = ps_a.tile([128, 512], F32, name=f"vpsa{b}_{jc}", tag="a")
                for kc in range(KC):
                    _mm(nc, vps_a, xs[kc], wvf[kc][:, 0:512],
                        start=(kc == 0), stop=(kc == KC - 1))
                nc.vector.tensor_copy(
                    out=vat[:, 0:8, 0:D],
                    in_=vps_a.rearrange("p (h d) -> p h d", h=8),
                )
                vps_b = ps_a.tile([128, 512], F32, name=f"vpsb{b}_{jc}", tag="a")
                for kc in range(KC):
                    _mm(nc, vps_b[:, 0:256], xs[kc], wvf[kc][:, 512:C],
                        start=(kc == 0), stop=(kc == KC - 1))
                nc.vector.tensor_copy(
                    out=vat[:, 8:H, 0:D],
                    in_=vps_b[:, 0:256].rearrange("p (h d) -> p h d", h=4),
                )
                nc.vector.memset(vat[:, :, D:D + 1], 1.0)
                va[(b, jc)] = vat
            return emit

        def qk_unit(which, mt):
            def emit():
                # k (which=1) is only ever a stationary operand of QK^T:
                # store it as fp8e4 so the per-matmul weight load halves.
                # One-sided k quantization error (~2.5% rms) only perturbs
                # the softmax logits by ~0.006 abs -> ~0.6% on P.
                dt = FP8 if which == 1 else BF16
                dest = qk_pool.tile([128, N], dt, name=f"qk{b}_{which}_{mt}", tag="qk")
                for hf in range(2):
                    ps = ps_a.tile([128, 512], F32, name=f"ps{b}_{which}_{mt}_{hf}", tag="a")
                    for kc in range(KC):
                        w = wqk[(which * KC + mt, kc)]
                        _mm(nc, ps,
                            w, xt[(b, kc)][:, hf * 512:(hf + 1) * 512],
                            start=(kc == 0), stop=(kc == KC - 1))
                    nc.vector.tensor_copy(out=dest[:, hf * 512:(hf + 1) * 512], in_=ps)
                (qt if which == 0 else kt)[(b, mt)] = dest
            return emit

        for jc in range(JC):
            units[("v", jc)] = v_unit(jc)
        for mt in range(KC):
            units[("k", mt)] = qk_unit(1, mt)
            units[("q", mt)] = qk_unit(0, mt)
        return units

    # ---- attention for batch b, with filler units paced between heads ----
    def qkt(b, h, jc):
        mt, off = h // 2, (h % 2) * D
        s = ps_s.tile([128, N], F32, name=f"s{b}_{h}_{jc}", tag="s")
        for hf in range(2):
            _mm(nc, s[:, hf * 512:(hf + 1) * 512],
                kt[(b, mt)][off:off + D, jc * 128:(jc + 1) * 128],
                qt[(b, mt)][off:off + D, hf * 512:(hf + 1) * 512])
        return s

    def attention(b, fillers, deadline_only=False):
        # fillers: list of (deadline_point, fn).  Units are emitted in list
        # order, spread evenly over the 8*H per-jc pace points; a unit
        # whose deadline arrives is forced out (with everything before it).
        # CRITICAL: pace() is called BEFORE each PV emission - the PE queue
        # is in-order, so filler matmuls must sit in front of the PV that
        # waits on its exp, or they cannot cover the ACT latency.
        emitted = 0
        npace = JC * H

        def pace(point):
            nonlocal emitted
            due = 0 if deadline_only else (
                ((point + 1) * len(fillers) + npace - 1) // npace)
            while emitted < len(fillers) and (
                emitted < due or fillers[emitted][0] <= point
            ):
                fillers[emitted][1]()
                emitted += 1

        s = qkt(b, 0, 0)
        for h in range(H):
            mt, off = h // 2, (h % 2) * D
            o_ps = [po_pool.tile([D + 1, 512], F32, name=f"ops{b}_{h}_{hf}", tag="ops")
                    for hf in range(2)]
            for jc in range(JC):
                pt = p_pool.tile([128, N], BF16, name=f"pt{b}_{h}_{jc}", tag="pt")
                nc.scalar.activation(out=pt, in_=s, func=EXP, scale=SCALE)
                if jc + 1 < JC:
                    s = qkt(b, h, jc + 1)
                elif h + 1 < H:
                    s = qkt(b, h + 1, 0)
                pace(JC * h + jc)
                for hf in range(2):
                    _mm(nc, o_ps[hf],
                        va[(b, jc)][:, h, :], pt[:, hf * 512:(hf + 1) * 512],
                        start=(jc == 0), stop=(jc == JC - 1))
            # normalize halves: rows 0..63 divided by l (= row 64)
            if (b, mt) not in ot:
                ot[(b, mt)] = ot_pool.tile([128, N], BF16, name=f"ot{b}_{mt}", tag="ot")
            # interleave the two halves' chains so the gpsimd broadcast of
            # half 0 overlaps the DVE copy/recip of half 1
            l_sbs, lbs = [], []
            for hf in range(2):
                l_sb = sm_pool.tile([1, 512], F32, name=f"l{b}_{h}_{hf}", tag="sm")
                nc.vector.tensor_copy(out=l_sb, in_=o_ps[hf][D:D + 1, :])
                nc.vector.reciprocal_approx_fast(out=l_sb, in_=l_sb)
                l_sbs.append(l_sb)
                lb = lb_pool.tile([D, 512], F32, name=f"lb{b}_{h}_{hf}", tag="lb")
                nc.gpsimd.partition_broadcast(lb, l_sb, channels=D)
                lbs.append(lb)
            for hf in range(2):
                nc.vector.tensor_mul(
                    out=ot[(b, mt)][off:off + D, hf * 512:(hf + 1) * 512],
                    in0=o_ps[hf][0:D, :], in1=lbs[hf],
                )

    # ---- output projection units for batch b (one unit per oc-half, so
    # the filler granularity is ~1.4us and every head boundary gets one) ----
    def outproj_units(b):
        units = []
        obs = {}

        def half_unit(oc, hf):
            def emit():
                if oc not in obs:
                    obs[oc] = ob_pool.tile([128, N], F32, name=f"ob{b}_{oc}", tag="ob")
                pps = ps_a.tile([128, 512], F32, name=f"pps{b}_{oc}_{hf}", tag="a")
                for kc in range(KC):
                    _mm(nc, pps,
                        wp[(kc, oc)],
                        ot[(b, kc)][:, hf * 512:(hf + 1) * 512],
                        start=(kc == 0), stop=(kc == KC - 1))
                nc.vector.tensor_scalar_add(
                    out=obs[oc][:, hf * 512:(hf + 1) * 512],
                    in0=pps, scalar1=bias_sb[:, oc:oc + 1])
                if hf == 1:
                    nc.sync.dma_start(out=outT_d[oc, b], in_=obs[oc])
            return emit

        for oc in range(KC):
            for hf in range(2):
                units.append(half_unit(oc, hf))
        return units

    # ---- pipeline ----
    # Window balance: the PE work of batch-1 qkv-proj and batch-0 out-proj
    # is spread across both attention windows so each window's PE work
    # slightly exceeds its ACT (exp) work and the PE never goes idle.
    # qk units for head-pair mt are consumed by qkt(h=2mt, 0), prefetched
    # at pace point 2*(2mt-1)+1; deadline one point earlier.
    a0 = phase_a_units(0)
    a1 = phase_a_units(1)
    LAST = JC * H - 1
    for key in [("k", 0), ("q", 0)] + [("v", jc) for jc in range(JC)] + [
        ("k", 1), ("q", 1), ("k", 2), ("q", 2), ("k", 3), ("q", 3)
    ]:
        a0[key]()
    fill0 = []
    for mt in (4, 5):  # batch-0 tail projections, hard deadlines
        dl = JC * (2 * mt - 1) - 1
        fill0 += [(dl, a0[("k", mt)]), (dl, a0[("q", mt)])]
    fill0 += [(LAST, a1[("v", jc)]) for jc in range(JC)]
    fill0 += [(LAST, a1[(w, 0)]) for w in ("k", "q")]
    attention(0, fill0)
    fill1 = []
    for mt in (1, 2, 3, 4, 5):
        dl = max(JC * (2 * mt - 1) - 1, 0)
        fill1 += [(dl, a1[("k", mt)]), (dl, a1[("q", mt)])]
    for i, u in enumerate(outproj_units(0)):
        fill1.append((JC * i + 3, u))
    fill1.sort(key=lambda t: t[0])
    attention(1, fill1, deadline_only=True)
    for u in outproj_units(1):
        u()


def get_nc():
    if "nc" not in _CACHE:
        nc = bacc.Bacc(None, target_bir_lowering=False, debug=False)
        with tile.TileContext(nc) as tc:
            with ExitStack() as ctx:
                _build(ctx, tc)
        nc.compile()
        _CACHE["nc"] = nc
    return _CACHE["nc"]


def make_in_maps(x, w_qkv, w_proj, b_proj):
    x = np.asarray(x, dtype=np.float32)
    w_qkv = np.asarray(w_qkv, dtype=np.float32)
    w_proj = np.asarray(w_proj, dtype=np.float32)
    # q/k weights j-major: wqkjb[j, p, kc*128+c2] = w_qkv[j*128+c2, kc*128+p]
    wqkj = np.ascontiguousarray(
        w_qkv[0:2 * C].reshape(2 * KC, 128, KC, 128).transpose(0, 3, 2, 1)
        .reshape(2 * KC, 128, C)
    ).astype(ml_dtypes.bfloat16)
    # v weights per-kc slabs [kc, 128p, 768]: w_qkv^T[c, 2C:3C]
    wvb = np.ascontiguousarray(
        w_qkv[2 * C:3 * C].T.reshape(KC, 128, C)
    ).astype(ml_dtypes.bfloat16)
    # w_proj^T [c, o] -> per-kc slabs [kc, 128, 768] bf16
    wprojb = np.ascontiguousarray(w_proj.T.reshape(KC, 128, C)).astype(ml_dtypes.bfloat16)
    bp = np.ascontiguousarray(b_proj.astype(np.float32).reshape(C, 1))
    in_maps = []
    for c in range(NCORES):
        # x^T [c, t] -> blocks [kc, b, 128, N] bf16
        xT = x[c * BL:(c + 1) * BL].reshape(T, C).T  # [768, 2048]
        xb = np.ascontiguousarray(
            xT.reshape(KC, 128, BL, N).transpose(0, 2, 1, 3)
        ).astype(ml_dtypes.bfloat16)
        in_maps.append({"xTb": xb, "wqkjb": wqkj, "wvb": wvb,
                        "wprojb": wprojb, "bproj": bp})
    return in_maps


def assemble_out(results):
    outs = []
    for c in range(NCORES):
        ob = results[c]["outTb"]  # [oc, b, 128, N]
        oT = ob.transpose(0, 2, 1, 3).reshape(C, T)
        outs.append(np.ascontiguousarray(oT.T).reshape(BL, N, C))
    return np.concatenate(outs, axis=0).astype(np.float32)


def kernel(x, w_qkv, w_proj, b_proj):
    nc = get_nc()
    in_maps = make_in_maps(x, w_qkv, w_proj, b_proj)
    res = bass_utils.run_bass_kernel_spmd(nc, in_maps, core_ids=list(range(NCORES)))
    return assemble_out(res.results)
